# revision 1
# baseline (speedup 1.0000x reference)
"""Self-contained Trainium2 Bass kernel for a 6-layer dense transformer.

Model (from reference): DIM=1024, DEPTH=6, HEADS=16, FF=4096, x [2,1024,1024],
relative_position_bias [1,16,1024,1024], pre-norm attention+FFN, exact GELU.

Strategy: sequence-parallel over 8 NeuronCores. Rows = flatten(batch, seq) =
2048; each core owns 256 rows (batch b = core//4, seq chunk core%4). Weights
are fully replicated and streamed from HBM each layer as bf16 via casting
gpsimd DMA, host-prepacked so every weight chunk is one contiguous 1 MiB read.
Activations are kept CHANNEL-major (transposed: [D, rows]) so every matmul
contraction lands on the partition axis with zero on-chip transposes:
  - LN stats via ones-vector matmuls (partition reduction on PE)
  - qkv^T = w_qkv.T @ h^T (weights as stationary lhsT, natural layout)
  - v computed row-major directly by swapping matmul operands
  - scores^T = k^T.T @ q^T per head; softmax without max-subtraction (scores
    are provably small); bias folded in as attn = exp(s)*exp(bias) with
    exp(bias) precomputed once in SBUF (bf16)
  - denominator via a ones-column appended to V (output row 64 of AV matmul)
  - per-layer AllGathers (bf16, split K then V for earlier overlap) across
    the 4 cores of each batch
Output is returned per-core as x^T shard; host assembles the full array.
"""
import sys
sys.path.insert(0, "/opt/trn_rl_repo")

import numpy as np

import concourse.bass as bass
import concourse.tile as tile
from concourse import bacc, mybir

P = 128
D = 1024
DT = 8            # D / P tiles
DEPTH = 6
HEADS = 16
DH = 64
FF = 4096
FFT = 32          # FF / P tiles
R = 256           # rows per core
B = 2
SEQ = 1024
N_CORES = 8
EPS = 1e-5
SCALE = DH ** -0.5
RG = [[0, 1, 2, 3], [4, 5, 6, 7]]

F32 = mybir.dt.float32
BF16 = mybir.dt.bfloat16
AX = mybir.AluOpType
AF = mybir.ActivationFunctionType

KV_K = D * R          # elems in k^T bounce
NQKV_CH = 12          # 256-col chunks in w_qkv
NOUT_CH = 4
NW1_CH = 16


def _bcast_mid(ap, n):
    """View a [P, N] AP as [P, n, N] with a 0-stride middle dim."""
    return bass.AP(tensor=ap.tensor, offset=ap.offset,
                   ap=[list(ap.ap[0]), [0, n], list(ap.ap[1])])


def build_nc(repeat=1):
    nc = bacc.Bacc("TRN2", target_bir_lowering=False, debug=False,
                   num_devices=N_CORES)

    xT_ext = nc.dram_tensor("xT", [D, R], F32, kind="ExternalInput")
    biasT_ext = nc.dram_tensor("biasT", [HEADS, 2, P, DT // 2, R], F32,
                               kind="ExternalInput")
    w_qkv_ext = nc.dram_tensor("w_qkv", [DEPTH, NQKV_CH, P, DT, 2 * P], F32,
                               kind="ExternalInput")
    w_out_ext = nc.dram_tensor("w_out", [DEPTH, NOUT_CH, P, DT, 2 * P], F32,
                               kind="ExternalInput")
    w1_ext = nc.dram_tensor("w1", [DEPTH, NW1_CH, P, DT, 2 * P], F32,
                            kind="ExternalInput")
    w2_ext = nc.dram_tensor("w2", [DEPTH, 4, 4, P, DT, 2 * P], F32,
                            kind="ExternalInput")
    b_out_ext = nc.dram_tensor("b_out", [DEPTH, D], F32, kind="ExternalInput")
    ln1_g_ext = nc.dram_tensor("ln1_g", [DEPTH, D], F32, kind="ExternalInput")
    ln1_b_ext = nc.dram_tensor("ln1_b", [DEPTH, D], F32, kind="ExternalInput")
    ln2_g_ext = nc.dram_tensor("ln2_g", [DEPTH, D], F32, kind="ExternalInput")
    ln2_b_ext = nc.dram_tensor("ln2_b", [DEPTH, D], F32, kind="ExternalInput")
    b1_ext = nc.dram_tensor("b1", [DEPTH, FF], F32, kind="ExternalInput")
    b2_ext = nc.dram_tensor("b2", [DEPTH, D], F32, kind="ExternalInput")
    outT_ext = nc.dram_tensor("outT", [D, R], F32, kind="ExternalOutput")

    from contextlib import ExitStack
    with tile.TileContext(nc) as tc, ExitStack() as ctx:
        ep = ctx.enter_context
        singles = ep(tc.tile_pool(name="singles", bufs=1))
        params = ep(tc.tile_pool(name="params", bufs=2))
        statp = ep(tc.tile_pool(name="stat", bufs=2))
        hTp = ep(tc.tile_pool(name="hTp", bufs=1))
        qTp = ep(tc.tile_pool(name="qTp", bufs=1))
        kvst = ep(tc.tile_pool(name="kvst", bufs=2))
        ktp = ep(tc.tile_pool(name="ktp", bufs=1))
        vpp = ep(tc.tile_pool(name="vpp", bufs=1))
        attnp = ep(tc.tile_pool(name="attnp", bufs=2))
        oTp = ep(tc.tile_pool(name="oTp", bufs=1))
        gTp = ep(tc.tile_pool(name="gTp", bufs=1))
        wcp = ep(tc.tile_pool(name="wcp", bufs=6))
        vecp = ep(tc.tile_pool(name="vecp", bufs=4))
        psmm = ep(tc.tile_pool(name="psmm", bufs=3, space="PSUM"))
        psav = ep(tc.tile_pool(name="psav", bufs=2, space="PSUM"))
        psbc = ep(tc.tile_pool(name="psbc", bufs=2, space="PSUM"))
        psst = ep(tc.tile_pool(name="psst", bufs=1, space="PSUM"))
        dram = ep(tc.tile_pool(name="dram", bufs=2, space="DRAM"))
        if True:
            # ---- persistent tiles ----
            xT = singles.tile([P, DT, R], F32, tag="xT")
            EB = singles.tile([P, HEADS, DT, R], BF16, tag="EB")
            ones_red = singles.tile([P, 1], BF16, tag="ones_red")
            ones_k1 = singles.tile([1, P], BF16, tag="ones_k1")
            nc.vector.memset(ones_red[:], 1.0)
            nc.vector.memset(ones_k1[:], 1.0)

            nc.sync.dma_start(
                out=xT[:], in_=xT_ext.ap().rearrange("(t p) r -> p t r", p=P))

            # EB = exp(bias^T) resident bf16 -- emitted inside layer 0 to
            # overlap the first AllGather / weight streaming.
            eb_emitted = [False]

            def emit_eb_load():
                if eb_emitted[0]:
                    return
                eb_emitted[0] = True
                for h in range(HEADS):
                    for hf in range(2):
                        tmp = statp.tile([P, DT // 2, R], F32, tag="stat",
                                         name=f"btmp_{h}_{hf}")
                        nc.sync.dma_start(out=tmp[:], in_=biasT_ext.ap()[h, hf])
                        nc.scalar.activation(EB[:, h, hf * 4:(hf + 1) * 4],
                                             tmp[:], AF.Exp)

            def ln_alloc(tag):
                xb = statp.tile([P, DT, R], BF16, tag="stat", name=f"xb_{tag}")
                sq = statp.tile([P, DT, R], BF16, tag="stat", name=f"sq_{tag}")
                ps_st = psst.tile([33, R], F32, tag="st", name=f"st_{tag}")
                return xb, sq, ps_st

            def ln_contrib(st, t):
                """Accumulate LN stats for channel-tile t of xT."""
                xb, sq, ps_st = st
                nc.vector.tensor_copy(xb[:, t], xT[:, t])
                nc.vector.tensor_mul(sq[:, t], xb[:, t], xb[:, t])
                nc.tensor.matmul(ps_st[0:1], ones_red[:], xb[:, t],
                                 start=(t == 0), stop=(t == DT - 1))
                nc.tensor.matmul(ps_st[32:33], ones_red[:], sq[:, t],
                                 start=(t == 0), stop=(t == DT - 1))

            def ln_finish(st, g_sb, b_sb, out_hT, tag):
                """LN over channel (partition) axis of xT -> out_hT (bf16)."""
                xb, sq, ps_st = st
                mu = vecp.tile([1, R], F32, tag="vec", name=f"mu_{tag}")
                var = vecp.tile([1, R], F32, tag="vec", name=f"var_{tag}")
                ms = vecp.tile([1, R], F32, tag="vec", name=f"ms_{tag}")
                rstd = vecp.tile([1, R], F32, tag="vec", name=f"rstd_{tag}")
                nc.vector.tensor_scalar_mul(mu[:], ps_st[0:1], 1.0 / D)
                nc.vector.tensor_scalar_mul(var[:], ps_st[32:33], 1.0 / D)
                nc.vector.tensor_mul(ms[:], mu[:], mu[:])
                nc.vector.tensor_sub(var[:], var[:], ms[:])
                nc.vector.tensor_scalar_add(var[:], var[:], EPS)
                nc.scalar.activation(var[:], var[:], AF.Sqrt)
                nc.vector.reciprocal(rstd[:], var[:])
                ones_f = vecp.tile([1, P], F32, tag="vec16", name=f"onesf_{tag}")
                nc.vector.memset(ones_f[:], 1.0)
                ps_mu = psbc.tile([P, R], F32, tag="bc", name=f"psmu_{tag}")
                ps_rs = psbc.tile([P, R], F32, tag="bc", name=f"psrs_{tag}")
                nc.tensor.matmul(ps_mu[:], ones_f[:], mu[:], start=True, stop=True)
                nc.tensor.matmul(ps_rs[:], ones_f[:], rstd[:], start=True, stop=True)
                # stage broadcasts in bf16 SBUF so the wide apply runs in the
                # DVE fast mode instead of 1x PSUM-read mode
                mub = statp.tile([P, R], BF16, tag="statv", name=f"mub_{tag}")
                rsb = statp.tile([P, R], BF16, tag="statv", name=f"rsb_{tag}")
                nc.vector.tensor_copy(mub[:], ps_mu[:])
                nc.vector.tensor_copy(rsb[:], ps_rs[:])
                nc.vector.tensor_sub(xb[:], xT[:], _bcast_mid(mub[:], DT))
                nc.vector.tensor_mul(xb[:], xb[:], _bcast_mid(rsb[:], DT))
                for t in range(DT):
                    nc.vector.tensor_scalar(
                        out_hT[:, t], xb[:, t], g_sb[:, t:t + 1], b_sb[:, t:t + 1],
                        op0=AX.mult, op1=AX.add)

            for _rep in range(repeat):
                for l in range(DEPTH):
                    g1 = params.tile([P, DT], F32, tag="g1")
                    b1p = params.tile([P, DT], F32, tag="b1p")
                    g2 = params.tile([P, DT], F32, tag="g2")
                    b2p = params.tile([P, DT], F32, tag="b2p")
                    bo = params.tile([P, DT], F32, tag="bo")
                    bf = params.tile([P, FFT], F32, tag="bf")
                    b2f = params.tile([P, DT], F32, tag="b2f")
                    nc.sync.dma_start(out=g1[:], in_=ln1_g_ext.ap()[l].rearrange("(t p) -> p t", p=P))
                    nc.sync.dma_start(out=b1p[:], in_=ln1_b_ext.ap()[l].rearrange("(t p) -> p t", p=P))
                    nc.sync.dma_start(out=g2[:], in_=ln2_g_ext.ap()[l].rearrange("(t p) -> p t", p=P))
                    nc.sync.dma_start(out=b2p[:], in_=ln2_b_ext.ap()[l].rearrange("(t p) -> p t", p=P))
                    nc.sync.dma_start(out=bo[:], in_=b_out_ext.ap()[l].rearrange("(t p) -> p t", p=P))
                    nc.sync.dma_start(out=bf[:], in_=b1_ext.ap()[l].rearrange("(t p) -> p t", p=P))
                    nc.sync.dma_start(out=b2f[:], in_=b2_ext.ap()[l].rearrange("(t p) -> p t", p=P))

                    # ---- LN1 (stats carried from prev mm2 epilogue) ----
                    if l == 0:
                        ln1_st = ln_alloc("l0a")
                        for t in range(DT):
                            ln_contrib(ln1_st, t)
                    hT = hTp.tile([P, DT, R], BF16, tag="hT", name=f"hT_{l}")
                    ln_finish(ln1_st, g1, b1p, hT, f"l{l}a")

                    qT = qTp.tile([P, DT, R], BF16, tag="qT", name=f"qT_{l}")
                    kst = kvst.tile([P, DT, R], BF16, tag="kv", name=f"kst_{l}")
                    vst = kvst.tile([P, 2, D], BF16, tag="kv", name=f"vst_{l}")

                    kv_in = dram.tile([2 * KV_K], BF16, tag="kv_in", name=f"kvi_{l}")
                    kv_out = dram.tile([4, 2 * KV_K], BF16, tag="kv_out", name=f"kvo_{l}")

                    # ---- QKV: k cols first, then v, then q (AGs early) ----
                    for ch in range(4, 8):      # k cols 1024..2047
                        wc = wcp.tile([P, DT, 2 * P], BF16, tag="wc",
                                      name=f"wck_{l}_{ch}")
                        nc.gpsimd.dma_start(out=wc[:], in_=w_qkv_ext.ap()[l, ch])
                        for sub in range(2):
                            c = ch * 2 + sub - 8
                            ps = psmm.tile([P, R], F32, tag="mm", name=f"psk_{l}_{ch}_{sub}")
                            for kt in range(DT):
                                nc.tensor.matmul(ps[:], wc[:, kt, sub * P:(sub + 1) * P],
                                                 hT[:, kt], start=(kt == 0), stop=(kt == DT - 1))
                            nc.vector.tensor_copy(kst[:, c], ps[:])
                            nc.sync.dma_start(
                                out=kv_in[c * P * R:(c + 1) * P * R].rearrange(
                                    "(p r) -> p r", p=P),
                                in_=kst[:, c])

                    for ch in range(8, 12):     # v cols 2048..3071 (row-major out)
                        wc = wcp.tile([P, DT, 2 * P], BF16, tag="wc",
                                      name=f"wcv_{l}_{ch}")
                        nc.gpsimd.dma_start(out=wc[:], in_=w_qkv_ext.ap()[l, ch])
                        for rt in range(2):
                            ps = psmm.tile([P, R], F32, tag="mm", name=f"psv_{l}_{ch}_{rt}")
                            for kt in range(DT):
                                nc.tensor.matmul(ps[:], hT[:, kt, rt * P:(rt + 1) * P],
                                                 wc[:, kt], start=(kt == 0), stop=(kt == DT - 1))
                            nc.vector.tensor_copy(
                                vst[:, rt, (ch - 8) * 256:(ch - 7) * 256], ps[:])
                            off = KV_K + rt * (P * D) + (ch - 8) * 256
                            nc.sync.dma_start(
                                out=bass.AP(tensor=kv_in[:].tensor,
                                            offset=kv_in[:].offset + off,
                                            ap=[[D, P], [1, 256]]),
                                in_=vst[:, rt, (ch - 8) * 256:(ch - 7) * 256])
                    nc.gpsimd.collective_compute(
                        "AllGather", AX.bypass, replica_groups=RG,
                        ins=[kv_in[:]], outs=[kv_out[:]])

                    for ch in range(4):         # q cols 0..1023 (overlaps AGs)
                        wc = wcp.tile([P, DT, 2 * P], BF16, tag="wc",
                                      name=f"wcq_{l}_{ch}")
                        nc.gpsimd.dma_start(out=wc[:], in_=w_qkv_ext.ap()[l, ch])
                        for sub in range(2):
                            c = ch * 2 + sub
                            ps = psmm.tile([P, R], F32, tag="mm", name=f"psq_{l}_{ch}_{sub}")
                            for kt in range(DT):
                                nc.tensor.matmul(ps[:], wc[:, kt, sub * P:(sub + 1) * P],
                                                 hT[:, kt], start=(kt == 0), stop=(kt == DT - 1))
                            nc.vector.tensor_copy(qT[:, c], ps[:])

                    # gathered K^T / V+ones into SBUF
                    KT2 = ktp.tile([P, DT, SEQ], BF16, tag="KT2", name=f"KT2_{l}")
                    Vp = vpp.tile([P, DT, HEADS, DH + 1], BF16, tag="Vp", name=f"Vp_{l}")
                    nc.vector.memset(Vp[:, :, :, DH:DH + 1], 1.0)
                    for r in range(4):
                        nc.sync.dma_start(
                            out=KT2[:, :, r * R:(r + 1) * R],
                            in_=kv_out[r, :KV_K].rearrange("(hp p k) -> p hp k",
                                                           p=P, k=R))
                        for t in range(2):
                            seg = KV_K + t * (P * D)
                            nc.sync.dma_start(
                                out=Vp[:, r * 2 + t, :, 0:DH],
                                in_=kv_out[r, seg:seg + P * D].rearrange(
                                    "(p h j) -> p h j", p=P, j=DH))

                    emit_eb_load()

                    # ---- attention per head ----
                    oT = oTp.tile([P, DT, R], BF16, tag="oT", name=f"oT_{l}")
                    for h in range(HEADS):
                        pb = (h % 2) * DH
                        at = attnp.tile([P, DT, R], BF16, tag="attn", name=f"at_{l}_{h}")
                        ps_o = psav.tile([DH + 1, R], F32, tag="av", name=f"pso_{l}_{h}")
                        for k2 in range(4):
                            ps_s = psmm.tile([P, 2 * R], F32, tag="mm",
                                             name=f"pss_{l}_{h}_{k2}")
                            for j in range(2):
                                kt = k2 * 2 + j
                                nc.tensor.matmul(
                                    ps_s[:, j * R:(j + 1) * R],
                                    KT2[pb:pb + DH, h // 2, kt * P:(kt + 1) * P],
                                    qT[pb:pb + DH, h // 2],
                                    start=True, stop=True)
                            nc.scalar.activation(
                                at[:, k2 * 2:(k2 + 1) * 2].rearrange("p a b -> p (a b)"),
                                ps_s[:], AF.Exp, scale=SCALE)
                            nc.vector.tensor_mul(
                                at[:, k2 * 2:(k2 + 1) * 2],
                                at[:, k2 * 2:(k2 + 1) * 2],
                                EB[:, h, k2 * 2:(k2 + 1) * 2])
                            for j in range(2):
                                kt = k2 * 2 + j
                                nc.tensor.matmul(ps_o[:], Vp[:, kt, h], at[:, kt],
                                                 start=(kt == 0), stop=(kt == DT - 1))
                        rec = vecp.tile([1, R], F32, tag="vec", name=f"rec_{l}_{h}")
                        rec16 = vecp.tile([1, R], BF16, tag="vec16", name=f"rec16_{l}_{h}")
                        nc.vector.reciprocal(rec[:], ps_o[DH:DH + 1])
                        nc.vector.tensor_copy(rec16[:], rec[:])
                        ps_b = psbc.tile([P, R], F32, tag="bc", name=f"ps_b_{l}_{h}")
                        nc.tensor.matmul(ps_b[0:DH], ones_k1[0:1, 0:DH], rec16[:],
                                         start=True, stop=True)
                        nc.vector.tensor_copy(oT[pb:pb + DH, h // 2], ps_o[0:DH])
                        nc.vector.tensor_mul(oT[pb:pb + DH, h // 2],
                                             oT[pb:pb + DH, h // 2], ps_b[0:DH])

                    # ---- attn out projection + residual (+LN2 stats) ----
                    ln2_st = ln_alloc(f"l{l}b")
                    for ch in range(NOUT_CH):
                        wc = wcp.tile([P, DT, 2 * P], BF16, tag="wc",
                                      name=f"wco_{l}_{ch}")
                        nc.gpsimd.dma_start(out=wc[:], in_=w_out_ext.ap()[l, ch])
                        for sub in range(2):
                            c = ch * 2 + sub
                            ps = psmm.tile([P, R], F32, tag="mm", name=f"pso2_{l}_{ch}_{sub}")
                            for kt in range(DT):
                                nc.tensor.matmul(ps[:], wc[:, kt, sub * P:(sub + 1) * P],
                                                 oT[:, kt], start=(kt == 0), stop=(kt == DT - 1))
                            nc.vector.scalar_tensor_tensor(
                                out=xT[:, c], in0=ps[:], scalar=bo[:, c:c + 1],
                                in1=xT[:, c], op0=AX.add, op1=AX.add)
                            ln_contrib(ln2_st, c)

                    # ---- LN2 + FFN ----
                    h2 = hTp.tile([P, DT, R], BF16, tag="hT", name=f"h2_{l}")
                    ln_finish(ln2_st, g2, b2p, h2, f"l{l}b")

                    gT = gTp.tile([P, FFT, R], BF16, tag="gT", name=f"gT_{l}")
                    for ch in range(NW1_CH):
                        wc = wcp.tile([P, DT, 2 * P], BF16, tag="wc",
                                      name=f"wc1_{l}_{ch}")
                        nc.gpsimd.dma_start(out=wc[:], in_=w1_ext.ap()[l, ch])
                        for sub in range(2):
                            f = ch * 2 + sub
                            ps = psmm.tile([P, R], F32, tag="mm", name=f"psf_{l}_{ch}_{sub}")
                            for kt in range(DT):
                                nc.tensor.matmul(ps[:], wc[:, kt, sub * P:(sub + 1) * P],
                                                 h2[:, kt], start=(kt == 0), stop=(kt == DT - 1))
                            nc.scalar.activation(gT[:, f], ps[:], AF.Gelu,
                                                 bias=bf[:, f:f + 1])

                    if l < DEPTH - 1:
                        ln1_st = ln_alloc(f"l{l + 1}a")
                    for cp in range(4):
                        pss = [psmm.tile([P, R], F32, tag="mm", name=f"ps_mm2_{l}_{cp}_{i}")
                               for i in range(2)]
                        for ktg in range(4):
                            wc = wcp.tile([P, DT, 2 * P], BF16, tag="wc",
                                          name=f"wc2_{l}_{cp}_{ktg}")
                            nc.gpsimd.dma_start(out=wc[:], in_=w2_ext.ap()[l, cp, ktg])
                            for sub in range(2):
                                for k8 in range(DT):
                                    nc.tensor.matmul(
                                        pss[sub][:], wc[:, k8, sub * P:(sub + 1) * P],
                                        gT[:, ktg * 8 + k8],
                                        start=(ktg == 0 and k8 == 0),
                                        stop=(ktg == 3 and k8 == DT - 1))
                        for sub in range(2):
                            c = cp * 2 + sub
                            nc.vector.scalar_tensor_tensor(
                                out=xT[:, c], in0=pss[sub][:], scalar=b2f[:, c:c + 1],
                                in1=xT[:, c], op0=AX.add, op1=AX.add)
                            if l < DEPTH - 1:
                                ln_contrib(ln1_st, c)

            nc.sync.dma_start(
                out=outT_ext.ap().rearrange("(t p) r -> p t r", p=P), in_=xT[:])

    nc.compile()
    return nc


def make_in_maps(inputs):
    x = np.ascontiguousarray(np.asarray(inputs["x"], dtype=np.float32))
    bias = np.asarray(inputs["relative_position_bias"], dtype=np.float32)

    def pack(w, nch):
        # [DEPTH, 128*DT rows, 256*nch cols] -> [DEPTH, nch, 128, DT, 256]
        w = np.asarray(w, dtype=np.float32)
        return np.ascontiguousarray(
            w.reshape(DEPTH, DT, P, nch, 2 * P).transpose(0, 3, 2, 1, 4))

    w2 = np.asarray(inputs["w2"], dtype=np.float32)
    w2p = np.ascontiguousarray(
        w2.reshape(DEPTH, 4, DT, P, 4, 2 * P).transpose(0, 4, 1, 3, 2, 5))

    shared = {
        "w_qkv": pack(inputs["w_qkv"], NQKV_CH),
        "w_out": pack(inputs["w_out"], NOUT_CH),
        "w1": pack(inputs["w1"], NW1_CH),
        "w2": w2p,
        "b_out": np.ascontiguousarray(inputs["b_out"], dtype=np.float32),
        "ln1_g": np.ascontiguousarray(inputs["ln1_g"], dtype=np.float32),
        "ln1_b": np.ascontiguousarray(inputs["ln1_b"], dtype=np.float32),
        "ln2_g": np.ascontiguousarray(inputs["ln2_g"], dtype=np.float32),
        "ln2_b": np.ascontiguousarray(inputs["ln2_b"], dtype=np.float32),
        "b1": np.ascontiguousarray(inputs["b1"], dtype=np.float32),
        "b2": np.ascontiguousarray(inputs["b2"], dtype=np.float32),
    }
    in_maps = []
    for c in range(N_CORES):
        b, s0 = c // 4, (c % 4) * R
        m = dict(shared)
        m["xT"] = np.ascontiguousarray(x[b, s0:s0 + R, :].T)
        bt = bias[0, :, s0:s0 + R, :].transpose(0, 2, 1)  # [16, 1024 keys, 256]
        m["biasT"] = np.ascontiguousarray(
            bt.reshape(HEADS, 2, DT // 2, P, R).transpose(0, 1, 3, 2, 4))
        in_maps.append(m)
    return in_maps


_NC_CACHE = {}


def kernel(**inputs):
    from concourse.bass_utils import run_bass_kernel_spmd
    if "nc" not in _NC_CACHE:
        _NC_CACHE["nc"] = build_nc()
    nc = _NC_CACHE["nc"]
    in_maps = make_in_maps(inputs)
    res = run_bass_kernel_spmd(nc, in_maps, core_ids=list(range(N_CORES)))
    out = np.empty((B, SEQ, D), dtype=np.float32)
    for c in range(N_CORES):
        b, s0 = c // 4, (c % 4) * R
        out[b, s0:s0 + R, :] = res.results[c]["outT"].T
    return out

